# revision 32
# baseline (speedup 1.0000x reference)
"""Multi-head attention (b=2, n=2048, dim=1024, h=16, fp32) on 8 TRN2 NeuronCores.

Sharding: 2 batches x 4 head-groups (4 heads each). Each core computes, for its
batch element and 4 heads: QKV projection, softmax attention, and a partial
output projection (W_out rows of its heads). Host sums the 4 partials per batch
and adds the bias.

Device layout choices (per core):
  - x arrives pre-transposed (host) as xT [128, 8, 2048] fp16 plus an fp8e4
    copy xT8; W_q/W_k arrive fp8e4, W_v/W_o fp16.
  - Q^T/K^T computed as [128, 2048] per head-pair via fp8 DoubleRow matmuls
    (contraction 256 per pass -> half the passes of fp16); V kept fp16 for
    precision (fp8 on the V path costs ~3.6% rel err, over budget).
  - S^T = K @ Q^T per head via row-tiled (K=64) fp16 matmul pairs; softmax exp
    on ScalarE directly PSUM->SBUF with scale=dim^-0.5 folded in (no max
    subtraction needed: |scores*scale| < ~0.5).
  - V is augmented with a ones column per head ([V_h | 1]) so the PV matmul's
    65th output row accumulates the softmax denominator for free.
  - Normalization: reciprocal_approx_fast (DVE) + partition_broadcast (GPSIMD)
    + one tensor_tensor multiply; the last sweep instead broadcasts via a PE
    rank-1 matmul (ones x recip) to cut tail latency.
  - Schedule: b-major sweeps with just-in-time projection fillers and
    out-projection chunks interleaved into later sweeps; kc-granular DMAs so
    the first matmuls start ~2us in; dummy warmup matmuls ramp the PE p-state
    during the initial DMA window.
"""

import os
import numpy as np
from contextlib import ExitStack
from collections import deque
from functools import partial

import concourse.bass as bass
import concourse.mybir as mybir
import concourse.tile as tile
from concourse import bacc
from concourse.bass import ts
from concourse.bass_utils import run_bass_kernel_spmd

F32 = mybir.dt.float32
F16 = mybir.dt.float16
F8 = mybir.dt.float8e4
DRMODE = mybir.MatmulPerfMode.DoubleRow

N_CORES = 8
HEADS = 16
DH = 64  # head dim


class Cfg:
    def __init__(self, n, dim, hg):
        self.n = n                    # sequence length (per core)
        self.dim = dim                # model dim
        self.hg = hg                  # heads per core
        self.kc = dim // 128          # dim chunks of 128
        self.nqb = max(1, n // 512)   # query blocks of 512
        self.qb = min(n, 512)
        self.nkc = n // 128           # key chunks of 128
        self.pairs = hg // 2
        self.shard = hg * DH          # qkv shard columns per section
        self.vw = hg * (DH + 1)       # V columns incl per-head ones col
        self.mm_dt = F16
        self.np_dt = np.float16
        self.qk_fp8 = os.environ.get("ATTN_QK_FP8", "1") == "1"


FULL = Cfg(2048, 1024, 4)


def build_kernel(tc, ctx, cfg, xT, xT8, wq, wk, wv, wo, out):
    nc = tc.nc
    P = 128
    KC, NQB, QB, NKC, PAIRS = cfg.kc, cfg.nqb, cfg.qb, cfg.nkc, cfg.pairs
    MD = cfg.mm_dt
    SCALE = cfg.dim ** -0.5
    M_SLABS = cfg.shard // 128  # = PAIRS
    QKDT = F8 if cfg.qk_fp8 else MD

    wpool = ctx.enter_context(tc.tile_pool(name="w", bufs=1))
    wq_sb = wpool.tile([P, KC, cfg.shard], QKDT, tag="wq", name="wq_sb")
    wk_sb = wpool.tile([P, KC, cfg.shard], QKDT, tag="wk", name="wk_sb")
    wv_sb = wpool.tile([P, KC, cfg.vw], MD, tag="wv", name="wv_sb")
    wo_sb = wpool.tile([P, M_SLABS, cfg.dim], MD, tag="wo", name="wo_sb")

    per = ctx.enter_context(tc.tile_pool(name="per", bufs=1))
    qt = {}  # (pair, nqb) -> [128, QB]
    kt = {}
    vt = {}  # nt -> [128, vw]
    on = {}  # (slab, nqb) -> [128, QB]  normalized O^T for out-proj lhsT
    for g in range(PAIRS):
        for b in range(NQB):
            qt[g, b] = per.tile([P, QB], MD, tag=f"qt{g}_{b}", name=f"qt{g}_{b}")
            kt[g, b] = per.tile([P, QB], MD, tag=f"kt{g}_{b}", name=f"kt{g}_{b}")
            on[g, b] = per.tile([P, QB], MD, tag=f"on{g}_{b}", name=f"on{g}_{b}")
    for t in range(NKC):
        vt[t] = per.tile([P, cfg.vw], MD, tag=f"v{t}", name=f"v{t}")

    xpool = ctx.enter_context(tc.tile_pool(name="x", bufs=1))
    paQ = ctx.enter_context(tc.tile_pool(name="paQ", bufs=2, space="PSUM"))
    psS = ctx.enter_context(tc.tile_pool(name="psS", bufs=2, space="PSUM"))
    psO = ctx.enter_context(tc.tile_pool(name="psO", bufs=1, space="PSUM"))
    epool = ctx.enter_context(tc.tile_pool(name="e", bufs=10))
    npool = ctx.enter_context(tc.tile_pool(name="nrm", bufs=3))
    copool = ctx.enter_context(tc.tile_pool(name="co", bufs=6))

    xts = {}
    x8s = {}
    for b in range(NQB):
        xts[b] = xpool.tile([P, KC, QB], MD, tag=f"xt{b}", name=f"xt{b}")
        if cfg.qk_fp8:
            x8s[b] = xpool.tile([P, KC, QB], F8, tag=f"x8{b}", name=f"x8{b}")
        else:
            x8s[b] = xts[b]

    # PE warmup: ~40 tiny matmuls on a zeroed tile ramp the tensor engine's
    # p-state while the first DMAs land.
    wt = xpool.tile([P, 64], MD, tag="warm", name="warm")
    nc.vector.memset(wt[:], 0.0)
    for i in range(16):
        ps = paQ.tile([P, 512], F32, tag="pa", name="warm_ps")
        nc.tensor.matmul(ps[0:64, 0:64], lhsT=wt[:], rhs=wt[:], start=True, stop=True)

    # DMA order follows first-use order: K weights + x8 block 0 (first K^T
    # emit), Q weights (Q^T), then x block 0 + V weights (V emits from c=0),
    # then the remaining blocks.
    h = KC // 2
    x80 = xT8 if cfg.qk_fp8 else xT
    nc.sync.dma_start(wk_sb[:, :h], wk[:, :h])
    nc.sync.dma_start(x8s[0][:, :h], x80[:, :h, ts(0, QB)])
    nc.sync.dma_start(wk_sb[:, h:], wk[:, h:])
    nc.sync.dma_start(x8s[0][:, h:], x80[:, h:, ts(0, QB)])
    nc.sync.dma_start(wq_sb[:], wq[:])
    if cfg.qk_fp8:
        nc.sync.dma_start(xts[0][:, :h], xT[:, :h, ts(0, QB)])
        nc.sync.dma_start(xts[0][:, h:], xT[:, h:, ts(0, QB)])
    nc.sync.dma_start(wv_sb[:, :h], wv[:, :h])
    nc.sync.dma_start(wv_sb[:, h:], wv[:, h:])
    for b in range(1, NQB):
        nc.sync.dma_start(xts[b][:, :h], xT[:, :h, ts(b, QB)])
        nc.sync.dma_start(xts[b][:, h:], xT[:, h:, ts(b, QB)])
        if cfg.qk_fp8:
            nc.sync.dma_start(x8s[b][:], xT8[:, :, ts(b, QB)])
    nc.sync.dma_start(wo_sb[:], wo[:])

    def emit_qk(w_sb, dst, g, b):
        # Q^T / K^T slab for head-pair g, query block b.
        ps = paQ.tile([P, 512], F32, tag="pa", name="pa")
        if cfg.qk_fp8:
            for j in range(KC // 2):
                nc.tensor.matmul(
                    ps[:, :QB],
                    lhsT=w_sb[:, 2 * j : 2 * j + 2, ts(g, 128)],
                    rhs=x8s[b][:, 2 * j : 2 * j + 2, :],
                    start=(j == 0),
                    stop=(j == KC // 2 - 1),
                    perf_mode=DRMODE,
                )
        else:
            for kc in range(KC):
                nc.tensor.matmul(
                    ps[:, :QB],
                    lhsT=w_sb[:, kc, ts(g, 128)],
                    rhs=x8s[b][:, kc, :],
                    start=(kc == 0),
                    stop=(kc == KC - 1),
                )
        nc.vector.tensor_copy(dst[g, b][:], ps[:, :QB])

    def emit_v(nt):
        vb, t = divmod(nt, QB // 128)
        ps = paQ.tile([P, 512], F32, tag="pa", name="pa")
        for kc in range(KC):
            nc.tensor.matmul(
                ps[:, : cfg.vw],
                lhsT=xts[vb][:, kc, ts(t, 128)],
                rhs=wv_sb[:, kc, :],
                start=(kc == 0),
                stop=(kc == KC - 1),
            )
        nc.vector.tensor_copy(vt[nt][:], ps[:, : cfg.vw])
        v4 = vt[nt][:].rearrange("p (h e) -> p h e", e=DH + 1)
        nc.vector.memset(v4[:, :, DH : DH + 1], 1.0)

    # ---- filler machinery: pending emissions pulled into sweeps ----
    pend = deque()        # big fillers: Q^T/K^T slab emissions (~1.8us each)
    pend_small = deque()  # small fillers: out-proj chunks (~0.5us each)
    emitted = set()

    def fill_one(q=None):
        key, fn = (q or pend).popleft()
        fn()
        emitted.add(key)

    def require(*keys):
        while pend and any(k not in emitted for k in keys):
            fill_one()

    ones_f32 = npool.tile([1, DH], F32, tag="ones", name="ones_f32")
    nc.vector.memset(ones_f32[:], 1.0)

    # ---- polynomial exp offload (DVE / GPSIMD) ----
    # e^y ~= C3*(y + R1)*(y^2 + P1*y + Q1), minimax cubic on y in [-0.65, 0.65]
    # (max rel err ~4e-3 through fp16; softmax normalization cancels most of
    # it since numerator and denominator share the same approximation).
    P1, Q1, R1, C3 = 1.3899519, 3.5932438, 1.6300839, 0.17061687
    ppool = ctx.enter_context(tc.tile_pool(name="poly", bufs=2))
    c3t = wpool.tile([P, 2, 512], MD, tag="c3t", name="c3t")
    nc.vector.memset(c3t[:], C3)
    ADD, MUL = mybir.AluOpType.add, mybir.AluOpType.mult

    def poly_exp(s_ps, e_t):
        # GPSIMD cannot read PSUM (the scale-cast runs on DVE) and supports
        # only plain tensor_tensor/tensor_scalar, so the cubic is 5 passes.
        g = nc.gpsimd
        y = ppool.tile([P, 2, 512], MD, tag="py", name="py")
        t1 = ppool.tile([P, 2, 512], MD, tag="pt1", name="pt1")
        u = ppool.tile([P, 2, 512], MD, tag="pu", name="pu")
        t = ppool.tile([P, 2, 512], MD, tag="pt", name="pt")
        nc.vector.tensor_scalar(out=y[:, :, :QB], in0=s_ps[:, :, :QB], scalar1=SCALE, scalar2=None, op0=MUL)
        g.tensor_scalar(out=t1[:, :, :QB], in0=y[:, :, :QB], scalar1=P1, scalar2=None, op0=ADD)
        g.tensor_tensor(u[:, :, :QB], t1[:, :, :QB], y[:, :, :QB], MUL)
        g.tensor_scalar(out=t[:, :, :QB], in0=y[:, :, :QB], scalar1=R1, scalar2=C3, op0=ADD, op1=MUL)
        g.tensor_scalar(out=t1[:, :, :QB], in0=u[:, :, :QB], scalar1=Q1, scalar2=None, op0=ADD)
        g.tensor_tensor(e_t[:, :, :QB], t1[:, :, :QB], t[:, :, :QB], MUL)

    def attention(b, g, with_v=False, slots=True, tail=False, offl=None):
        require(("q", g, b), ("k", g, 0))
        offl = offl or {}
        o_ps = psO.tile([P, 2, 512], F32, tag="o", name="o_ps")
        e_ts = {}
        pvq = deque()
        npv = [0]

        def emit_pv(c):
            v4 = vt[c][:].rearrange("p (h e) -> p h e", e=DH + 1)
            for a in range(2):
                h = 2 * g + a
                nc.tensor.matmul(
                    o_ps[0 : DH + 1, a, :QB],
                    lhsT=v4[:, h, :],
                    rhs=e_ts[c][:, a, :QB],
                    start=(npv[0] == 0),
                    stop=(npv[0] == NKC - 1),
                )
            npv[0] += 1

        for c in range(NKC):
            cb = c * 128 // QB
            if c == 4 * cb and cb > 0:
                require(("k", g, cb))
            s_ps = psS.tile([P, 2, 512], F32, tag="s", name="s_ps")
            for a in range(2):
                lo = a * 64
                nc.tensor.matmul(
                    s_ps[:, a, :QB],
                    lhsT=kt[g, cb][lo : lo + 64, ts(c % (QB // 128), 128)],
                    rhs=qt[g, b][lo : lo + 64, :],
                    start=True,
                    stop=True,
                )
            if with_v:
                emit_v(c)
            if slots:
                # small (out-proj) fillers slot in at any odd chunk; big
                # projection fillers only once per sweep to avoid bunching
                if pend_small and c % 2 == 1:
                    fill_one(pend_small)
                elif pend and c == 9:
                    fill_one()
            e_t = epool.tile([P, 2, 512], MD, tag="e", name="e_t")
            e_ts[c] = e_t
            if c in offl:
                poly_exp(s_ps, e_t)
            else:
                nc.scalar.activation(
                    e_t[:, :, :QB],
                    s_ps[:, :, :QB],
                    mybir.ActivationFunctionType.Exp,
                    scale=SCALE,
                )
            # PV for an offloaded chunk is deferred five chunks so the slower
            # GPSIMD pipeline does not stall the in-order PE queue.
            pvq.append(c)
            while pvq and (pvq[0] not in offl or c - pvq[0] >= 5):
                emit_pv(pvq.popleft())
        while pvq:
            emit_pv(pvq.popleft())
        # normalize; stage the denom row at partition 0 (the custom DVE
        # reciprocal misreads inputs at a nonzero base partition)
        if not tail:
            # o_ps must drain before the next sweep's first PV matmul (psO is
            # single-buffered and the PE queue is in-order): drow on DVE and
            # the main evacuation on GPSIMD run concurrently at sweep end.
            drow = npool.tile([1, 2, 512], F32, tag="drow", name="drow")
            nc.vector.tensor_copy(drow[:, :, :QB], o_ps[DH : DH + 1, :, :QB])
            oev = npool.tile([P, 2, 512], F32, tag="oev", name="oev")
            nc.scalar.copy(oev[0:DH, :, :QB], o_ps[0:DH, :, :QB])
            recip = npool.tile([1, 2, 512], F32, tag="recip", name="recip")
            nc.vector.reciprocal_approx_fast(out=recip[:, :, :QB], in_=drow[:, :, :QB])
            bcast = npool.tile([64, 2, 512], F32, tag="bcast", name="bcast")
            nc.gpsimd.partition_broadcast(bcast[:, :, :QB], recip[:, :, :QB])
            for a in range(2):
                nc.vector.tensor_tensor(
                    on[g, b][a * 64 : a * 64 + 64, :],
                    oev[0:DH, a, :QB],
                    bcast[:, a, :QB],
                    mybir.AluOpType.mult,
                )
            return
        # tail sweep: normalize in two query halves so the final out-proj
        # block starts while the second half is still normalizing; broadcast
        # via a PE rank-1 fp32 matmul (the PE is idle here, and it is much
        # lower latency than the GPSIMD partition_broadcast).
        HB = QB // 2
        oev = npool.tile([P, 2, 512], F32, tag="oev", name="oev")
        drow = npool.tile([1, 2, 512], F32, tag="drow", name="drow")
        recip = npool.tile([1, 2, 512], F32, tag="recip", name="recip")
        for hx in range(2):
            qs = slice(hx * HB, (hx + 1) * HB)
            nc.vector.tensor_copy(oev[0:DH, :, qs], o_ps[0:DH, :, qs])
            nc.vector.tensor_copy(drow[:, :, qs], o_ps[DH : DH + 1, :, qs])
            nc.vector.reciprocal_approx_fast(out=recip[:, :, qs], in_=drow[:, :, qs])
            for a in range(2):
                bc_ps = paQ.tile([P, 512], F32, tag="pa", name="bc_ps")
                nc.tensor.matmul(
                    bc_ps[0:DH, :HB],
                    lhsT=ones_f32[:],
                    rhs=recip[:, a, qs],
                    start=True,
                    stop=True,
                )
                nc.vector.tensor_tensor(
                    on[g, b][a * 64 : a * 64 + 64, qs],
                    oev[0:DH, a, qs],
                    bc_ps[0:DH, :HB],
                    mybir.AluOpType.mult,
                )

    NH = cfg.dim // 512
    ot_half = {}

    def out_proj_chunk(bb, t, nh, scalar_evac=False, pool=None):
        nt = bb * (QB // 128) + t
        if pool is None:
            ps = paQ.tile([P, 512], F32, tag="pa", name="pc")
        else:
            # tail: borrow idle psS/psO banks for a deeper psum pipeline
            ps = pool.tile([P, 2, 512], F32, tag=("s" if pool is psS else "o"), name="pc")[:, 0, :]
        for kc in range(M_SLABS):
            nc.tensor.matmul(
                ps[:],
                lhsT=on[kc, bb][:, ts(t, 128)],
                rhs=wo_sb[:, kc, ts(nh, 512)],
                start=(kc == 0),
                stop=(kc == M_SLABS - 1),
            )
        # both nh halves share one [128, 1024] tile; a single row-block DMA
        # fires with the second half (fewer DMAs -> less sem/teardown cost)
        if nh == 0:
            ot = copool.tile([P, 1024], MD, tag="ot", name="ot")
            ot_half[nt] = ot
        else:
            ot = ot_half.pop(nt)
        if scalar_evac:
            nc.scalar.copy(ot[:, ts(nh, 512)], ps[:])
        else:
            nc.vector.tensor_copy(ot[:, ts(nh, 512)], ps[:])
        if nh == NH - 1:
            nc.sync.dma_start(out[ts(nt, 128), :], ot[:])

    # ---- emission schedule (b-major: both head-pairs of a block, then the
    # next block; out-proj chunks for block b become fillers two sweeps on) ----
    emit_qk(wk_sb, kt, 0, 0)
    emitted.add(("k", 0, 0))
    emit_qk(wq_sb, qt, 0, 0)
    emitted.add(("q", 0, 0))

    for b in range(1, NQB):
        pend.append((("k", 0, b), partial(emit_qk, wk_sb, kt, 0, b)))
    for b in range(1, NQB):
        pend.append((("q", 0, b), partial(emit_qk, wq_sb, qt, 0, b)))
    pend.append((("q", 1, 0), partial(emit_qk, wq_sb, qt, 1, 0)))
    for b in range(NQB):
        pend.append((("k", 1, b), partial(emit_qk, wk_sb, kt, 1, b)))
    for b in range(1, NQB):
        pend.append((("q", 1, b), partial(emit_qk, wq_sb, qt, 1, b)))

    for g in range(PAIRS):
        for b in range(NQB):
            first = b == 0 and g == 0
            last = b == NQB - 1 and g == PAIRS - 1
            offl = {} if first else {4, 9}
            attention(b, g, with_v=first, slots=not first, tail=last, offl=offl)
            if g == PAIRS - 1 and b < NQB - 1:
                for t in range(QB // 128):
                    for nh in range(NH):
                        pend_small.append(
                            (("o", b, t, nh), partial(out_proj_chunk, b, t, nh))
                        )
    while pend:
        fill_one()
    while pend_small:
        fill_one(pend_small)
    # tail block: rotate through the now-idle psum pools for a deeper
    # pipeline, and alternate evac engines (ScalarE is idle after the last
    # exp)
    pools = [None, psS, psO]
    i = 0
    for t in range(QB // 128):
        for nh in range(NH):
            out_proj_chunk(NQB - 1, t, nh, scalar_evac=(nh == 0), pool=pools[i % 3])
            i += 1


def build_program(cfg, num_devices=N_CORES):
    nc = bacc.Bacc("TRN2", target_bir_lowering=False, debug=False, num_devices=num_devices)
    P = 128
    qkdt = F8 if cfg.qk_fp8 else cfg.mm_dt
    xT = nc.dram_tensor("xT", [P, cfg.kc, cfg.n], cfg.mm_dt, kind="ExternalInput").ap()
    xT8 = nc.dram_tensor("xT8", [P, cfg.kc, cfg.n], qkdt, kind="ExternalInput").ap()
    wq = nc.dram_tensor("wq", [P, cfg.kc, cfg.shard], qkdt, kind="ExternalInput").ap()
    wk = nc.dram_tensor("wk", [P, cfg.kc, cfg.shard], qkdt, kind="ExternalInput").ap()
    wv = nc.dram_tensor("wv", [P, cfg.kc, cfg.vw], cfg.mm_dt, kind="ExternalInput").ap()
    wo = nc.dram_tensor("wo", [P, cfg.shard // 128, cfg.dim], cfg.mm_dt, kind="ExternalInput").ap()
    out = nc.dram_tensor("out", [cfg.n, cfg.dim], cfg.mm_dt, kind="ExternalOutput").ap()
    with tile.TileContext(nc) as tc, ExitStack() as ctx:
        build_kernel(tc, ctx, cfg, xT, xT8, wq, wk, wv, wo, out)
    nc.compile()
    return nc


def shard_inputs(cfg, x, W_qkv, W_out, n_groups):
    """Build per-core input maps. Core c = (batch b, head-group g): c = b*n_groups + g."""
    import ml_dtypes

    f8 = ml_dtypes.float8_e4m3
    qk_dt = f8 if cfg.qk_fp8 else cfg.np_dt
    b_sz = x.shape[0]
    dim, hg, sh = cfg.dim, cfg.hg, cfg.shard
    xTs = []
    for b in range(b_sz):
        xt = np.ascontiguousarray(
            x[b].T.reshape(cfg.kc, 128, cfg.n).transpose(1, 0, 2)
        )
        xTs.append(xt)

    def wlayout(w):  # [dim, C] -> [128, kc, C]
        return np.ascontiguousarray(
            w.reshape(cfg.kc, 128, w.shape[1]).transpose(1, 0, 2)
        )

    in_maps = []
    for b in range(b_sz):
        for g in range(n_groups):
            wq = W_qkv[:, sh * g : sh * (g + 1)]
            wk = W_qkv[:, dim + sh * g : dim + sh * (g + 1)]
            wv_cols = W_qkv[:, 2 * dim + sh * g : 2 * dim + sh * (g + 1)]
            wv = np.zeros((dim, cfg.vw), np.float32)
            for h in range(hg):
                wv[:, h * (DH + 1) : h * (DH + 1) + DH] = wv_cols[:, h * DH : (h + 1) * DH]
            wo = W_out[sh * g : sh * (g + 1), :]
            wo_l = np.ascontiguousarray(
                wo.reshape(sh // 128, 128, dim).transpose(1, 0, 2)
            )
            in_maps.append(
                {
                    "xT": xTs[b].astype(cfg.np_dt),
                    "xT8": xTs[b].astype(qk_dt),
                    "wq": wlayout(wq).astype(qk_dt),
                    "wk": wlayout(wk).astype(qk_dt),
                    "wv": wlayout(wv).astype(cfg.np_dt),
                    "wo": wo_l.astype(cfg.np_dt),
                }
            )
    return in_maps


_NC_CACHE = {}


def kernel(x, W_qkv, W_out, b_out):
    x = np.asarray(x, np.float32)
    W_qkv = np.asarray(W_qkv, np.float32)
    W_out = np.asarray(W_out, np.float32)
    b_out = np.asarray(b_out, np.float32)
    cfg = FULL
    bsz = x.shape[0]
    n_groups = N_CORES // bsz

    if "nc" not in _NC_CACHE:
        _NC_CACHE["nc"] = build_program(cfg)
    nc = _NC_CACHE["nc"]

    in_maps = shard_inputs(cfg, x, W_qkv, W_out, n_groups)
    res = run_bass_kernel_spmd(nc, in_maps, list(range(N_CORES)))

    out = np.zeros((bsz, cfg.n, cfg.dim), np.float32)
    for b in range(bsz):
        for g in range(n_groups):
            out[b] += res.results[b * n_groups + g]["out"].astype(np.float32)
        out[b] += b_out
    return out


# revision 34
# speedup vs baseline: 3.0574x; 3.0574x over previous
"""Multi-head attention (b=2, n=2048, dim=1024, h=16, fp32) on 8 TRN2 NeuronCores.

Sharding: 2 batches x 4 head-groups (4 heads each). Each core computes, for its
batch element and 4 heads: QKV projection, softmax attention, and a partial
output projection (W_out rows of its heads). Host sums the 4 partials per batch
and adds the bias.

Device layout choices (per core):
  - x arrives pre-transposed (host) as xT [128, 8, 2048] fp16 plus an fp8e4
    copy xT8; W_q/W_k arrive fp8e4, W_v/W_o fp16.
  - Q^T/K^T computed as [128, 2048] per head-pair via fp8 DoubleRow matmuls
    (contraction 256 per pass -> half the passes of fp16); V kept fp16 for
    precision (fp8 on the V path costs ~3.6% rel err, over budget).
  - S^T = K @ Q^T per head via row-tiled (K=64) fp16 matmul pairs; softmax exp
    on ScalarE directly PSUM->SBUF with scale=dim^-0.5 folded in (no max
    subtraction needed: |scores*scale| < ~0.5).
  - V is augmented with a ones column per head ([V_h | 1]) so the PV matmul's
    65th output row accumulates the softmax denominator for free.
  - Normalization: reciprocal_approx_fast (DVE) + partition_broadcast (GPSIMD)
    + one tensor_tensor multiply; the last sweep instead broadcasts via a PE
    rank-1 matmul (ones x recip) to cut tail latency.
  - Schedule: b-major sweeps with just-in-time projection fillers and
    out-projection chunks interleaved into later sweeps; kc-granular DMAs so
    the first matmuls start ~2us in; dummy warmup matmuls ramp the PE p-state
    during the initial DMA window.
"""

import os
import numpy as np
from contextlib import ExitStack
from collections import deque
from functools import partial

import concourse.bass as bass
import concourse.mybir as mybir
import concourse.tile as tile
from concourse import bacc
from concourse.bass import ts
from concourse.bass_utils import run_bass_kernel_spmd

F32 = mybir.dt.float32
F16 = mybir.dt.float16
F8 = mybir.dt.float8e4
DRMODE = mybir.MatmulPerfMode.DoubleRow

N_CORES = 8
HEADS = 16
DH = 64  # head dim


class Cfg:
    def __init__(self, n, dim, hg):
        self.n = n                    # sequence length (per core)
        self.dim = dim                # model dim
        self.hg = hg                  # heads per core
        self.kc = dim // 128          # dim chunks of 128
        self.nqb = max(1, n // 512)   # query blocks of 512
        self.qb = min(n, 512)
        self.nkc = n // 128           # key chunks of 128
        self.pairs = hg // 2
        self.shard = hg * DH          # qkv shard columns per section
        self.vw = hg * (DH + 1)       # V columns incl per-head ones col
        self.mm_dt = F16
        self.np_dt = np.float16
        self.qk_fp8 = os.environ.get("ATTN_QK_FP8", "1") == "1"


FULL = Cfg(2048, 1024, 4)


def build_kernel(tc, ctx, cfg, xT, xT8, wq, wk, wv, wo, out):
    nc = tc.nc
    P = 128
    KC, NQB, QB, NKC, PAIRS = cfg.kc, cfg.nqb, cfg.qb, cfg.nkc, cfg.pairs
    MD = cfg.mm_dt
    SCALE = cfg.dim ** -0.5
    M_SLABS = cfg.shard // 128  # = PAIRS
    QKDT = F8 if cfg.qk_fp8 else MD

    wpool = ctx.enter_context(tc.tile_pool(name="w", bufs=1))
    wq_sb = wpool.tile([P, KC, cfg.shard], QKDT, tag="wq", name="wq_sb")
    wk_sb = wpool.tile([P, KC, cfg.shard], QKDT, tag="wk", name="wk_sb")
    wv_sb = wpool.tile([P, KC, cfg.vw], MD, tag="wv", name="wv_sb")
    wo_sb = wpool.tile([P, M_SLABS, cfg.dim], MD, tag="wo", name="wo_sb")

    per = ctx.enter_context(tc.tile_pool(name="per", bufs=1))
    qt = {}  # (pair, nqb) -> [128, QB]
    kt = {}
    vt = {}  # nt -> [128, vw]
    on = {}  # (slab, nqb) -> [128, QB]  normalized O^T for out-proj lhsT
    for g in range(PAIRS):
        for b in range(NQB):
            qt[g, b] = per.tile([P, QB], MD, tag=f"qt{g}_{b}", name=f"qt{g}_{b}")
            kt[g, b] = per.tile([P, QB], MD, tag=f"kt{g}_{b}", name=f"kt{g}_{b}")
            on[g, b] = per.tile([P, QB], MD, tag=f"on{g}_{b}", name=f"on{g}_{b}")
    for t in range(NKC):
        vt[t] = per.tile([P, cfg.vw], MD, tag=f"v{t}", name=f"v{t}")

    xpool = ctx.enter_context(tc.tile_pool(name="x", bufs=1))
    paQ = ctx.enter_context(tc.tile_pool(name="paQ", bufs=2, space="PSUM"))
    psS = ctx.enter_context(tc.tile_pool(name="psS", bufs=2, space="PSUM"))
    psO = ctx.enter_context(tc.tile_pool(name="psO", bufs=1, space="PSUM"))
    epool = ctx.enter_context(tc.tile_pool(name="e", bufs=10))
    npool = ctx.enter_context(tc.tile_pool(name="nrm", bufs=3))
    copool = ctx.enter_context(tc.tile_pool(name="co", bufs=6))

    xts = {}
    x8s = {}
    for b in range(NQB):
        xts[b] = xpool.tile([P, KC, QB], MD, tag=f"xt{b}", name=f"xt{b}")
        if cfg.qk_fp8:
            x8s[b] = xpool.tile([P, KC, QB], F8, tag=f"x8{b}", name=f"x8{b}")
        else:
            x8s[b] = xts[b]

    # PE warmup: ~40 tiny matmuls on a zeroed tile ramp the tensor engine's
    # p-state while the first DMAs land.
    wt = xpool.tile([P, 64], MD, tag="warm", name="warm")
    nc.vector.memset(wt[:], 0.0)
    for i in range(16):
        ps = paQ.tile([P, 512], F32, tag="pa", name="warm_ps")
        nc.tensor.matmul(ps[0:64, 0:64], lhsT=wt[:], rhs=wt[:], start=True, stop=True)

    # DMA order follows first-use order: K weights + x8 block 0 (first K^T
    # emit), Q weights (Q^T), then x block 0 + V weights (V emits from c=0),
    # then the remaining blocks.
    h = KC // 2
    x80 = xT8 if cfg.qk_fp8 else xT
    nc.sync.dma_start(wk_sb[:, :h], wk[:, :h])
    nc.sync.dma_start(x8s[0][:, :h], x80[:, :h, ts(0, QB)])
    nc.sync.dma_start(wk_sb[:, h:], wk[:, h:])
    nc.sync.dma_start(x8s[0][:, h:], x80[:, h:, ts(0, QB)])
    nc.sync.dma_start(wq_sb[:], wq[:])
    if cfg.qk_fp8:
        nc.sync.dma_start(xts[0][:, :h], xT[:, :h, ts(0, QB)])
        nc.sync.dma_start(xts[0][:, h:], xT[:, h:, ts(0, QB)])
    nc.sync.dma_start(wv_sb[:, :h], wv[:, :h])
    nc.sync.dma_start(wv_sb[:, h:], wv[:, h:])
    for b in range(1, NQB):
        nc.sync.dma_start(xts[b][:, :h], xT[:, :h, ts(b, QB)])
        nc.sync.dma_start(xts[b][:, h:], xT[:, h:, ts(b, QB)])
        if cfg.qk_fp8:
            nc.sync.dma_start(x8s[b][:], xT8[:, :, ts(b, QB)])
    nc.sync.dma_start(wo_sb[:], wo[:])

    def emit_qk(w_sb, dst, g, b):
        # Q^T / K^T slab for head-pair g, query block b.
        ps = paQ.tile([P, 512], F32, tag="pa", name="pa")
        if cfg.qk_fp8:
            for j in range(KC // 2):
                nc.tensor.matmul(
                    ps[:, :QB],
                    lhsT=w_sb[:, 2 * j : 2 * j + 2, ts(g, 128)],
                    rhs=x8s[b][:, 2 * j : 2 * j + 2, :],
                    start=(j == 0),
                    stop=(j == KC // 2 - 1),
                    perf_mode=DRMODE,
                )
        else:
            for kc in range(KC):
                nc.tensor.matmul(
                    ps[:, :QB],
                    lhsT=w_sb[:, kc, ts(g, 128)],
                    rhs=x8s[b][:, kc, :],
                    start=(kc == 0),
                    stop=(kc == KC - 1),
                )
        nc.vector.tensor_copy(dst[g, b][:], ps[:, :QB])

    def emit_v(nt):
        vb, t = divmod(nt, QB // 128)
        ps = paQ.tile([P, 512], F32, tag="pa", name="pa")
        for kc in range(KC):
            nc.tensor.matmul(
                ps[:, : cfg.vw],
                lhsT=xts[vb][:, kc, ts(t, 128)],
                rhs=wv_sb[:, kc, :],
                start=(kc == 0),
                stop=(kc == KC - 1),
            )
        nc.vector.tensor_copy(vt[nt][:], ps[:, : cfg.vw])
        v4 = vt[nt][:].rearrange("p (h e) -> p h e", e=DH + 1)
        nc.vector.memset(v4[:, :, DH : DH + 1], 1.0)

    # ---- filler machinery: pending emissions pulled into sweeps ----
    pend = deque()        # big fillers: Q^T/K^T slab emissions (~1.8us each)
    pend_small = deque()  # small fillers: out-proj chunks (~0.5us each)
    emitted = set()

    def fill_one(q=None):
        key, fn = (q or pend).popleft()
        fn()
        emitted.add(key)

    def require(*keys):
        while pend and any(k not in emitted for k in keys):
            fill_one()

    ones_f32 = npool.tile([1, DH], F32, tag="ones", name="ones_f32")
    nc.vector.memset(ones_f32[:], 1.0)

    # ---- polynomial exp offload (DVE / GPSIMD) ----
    # e^y ~= C3*(y + R1)*(y^2 + P1*y + Q1), minimax cubic on y in [-0.65, 0.65]
    # (max rel err ~4e-3 through fp16; softmax normalization cancels most of
    # it since numerator and denominator share the same approximation).
    P1, Q1, R1, C3 = 1.3899519, 3.5932438, 1.6300839, 0.17061687
    ppool = ctx.enter_context(tc.tile_pool(name="poly", bufs=2))
    c3t = wpool.tile([P, 2, 512], MD, tag="c3t", name="c3t")
    nc.vector.memset(c3t[:], C3)
    ADD, MUL = mybir.AluOpType.add, mybir.AluOpType.mult

    def poly_exp(s_ps, e_t):
        # GPSIMD cannot read PSUM (the scale-cast runs on DVE) and supports
        # only plain tensor_tensor/tensor_scalar, so the cubic is 5 passes.
        g = nc.gpsimd
        y = ppool.tile([P, 2, 512], MD, tag="py", name="py")
        t1 = ppool.tile([P, 2, 512], MD, tag="pt1", name="pt1")
        u = ppool.tile([P, 2, 512], MD, tag="pu", name="pu")
        t = ppool.tile([P, 2, 512], MD, tag="pt", name="pt")
        nc.vector.tensor_scalar(out=y[:, :, :QB], in0=s_ps[:, :, :QB], scalar1=SCALE, scalar2=None, op0=MUL)
        g.tensor_scalar(out=t1[:, :, :QB], in0=y[:, :, :QB], scalar1=P1, scalar2=None, op0=ADD)
        g.tensor_tensor(u[:, :, :QB], t1[:, :, :QB], y[:, :, :QB], MUL)
        g.tensor_scalar(out=t[:, :, :QB], in0=y[:, :, :QB], scalar1=R1, scalar2=C3, op0=ADD, op1=MUL)
        g.tensor_scalar(out=t1[:, :, :QB], in0=u[:, :, :QB], scalar1=Q1, scalar2=None, op0=ADD)
        g.tensor_tensor(e_t[:, :, :QB], t1[:, :, :QB], t[:, :, :QB], MUL)

    def attention(b, g, with_v=False, slots=True, tail=False, offl=None):
        require(("q", g, b), ("k", g, 0))
        offl = offl or {}
        o_ps = psO.tile([P, 2, 512], F32, tag="o", name="o_ps")
        e_ts = {}
        pvq = deque()
        npv = [0]

        def emit_pv(c):
            v4 = vt[c][:].rearrange("p (h e) -> p h e", e=DH + 1)
            for a in range(2):
                h = 2 * g + a
                nc.tensor.matmul(
                    o_ps[0 : DH + 1, a, :QB],
                    lhsT=v4[:, h, :],
                    rhs=e_ts[c][:, a, :QB],
                    start=(npv[0] == 0),
                    stop=(npv[0] == NKC - 1),
                )
            npv[0] += 1

        for c in range(NKC):
            cb = c * 128 // QB
            if c == 4 * cb and cb > 0:
                require(("k", g, cb))
            s_ps = psS.tile([P, 2, 512], F32, tag="s", name="s_ps")
            for a in range(2):
                lo = a * 64
                nc.tensor.matmul(
                    s_ps[:, a, :QB],
                    lhsT=kt[g, cb][lo : lo + 64, ts(c % (QB // 128), 128)],
                    rhs=qt[g, b][lo : lo + 64, :],
                    start=True,
                    stop=True,
                )
            if with_v:
                emit_v(c)
            if slots:
                # small (out-proj) fillers slot in at any odd chunk; big
                # projection fillers only once per sweep to avoid bunching
                if pend_small and c % 2 == 1:
                    fill_one(pend_small)
                elif pend and c == 9:
                    fill_one()
            e_t = epool.tile([P, 2, 512], MD, tag="e", name="e_t")
            e_ts[c] = e_t
            nc.scalar.activation(
                e_t[:, :, :QB],
                s_ps[:, :, :QB],
                mybir.ActivationFunctionType.Exp,
                scale=SCALE,
            )
            # PV runs two chunks behind S in the in-order PE queue: at sweep
            # start the PE then has S work in hand while the previous sweep's
            # o_ps evacuation (which gates this sweep's first PV) drains.
            pvq.append(c)
            while pvq and c - pvq[0] >= 2:
                emit_pv(pvq.popleft())
        while pvq:
            emit_pv(pvq.popleft())
        # normalize; stage the denom row at partition 0 (the custom DVE
        # reciprocal misreads inputs at a nonzero base partition)
        if not tail:
            # o_ps must drain before the next sweep's first PV matmul (psO is
            # single-buffered and the PE queue is in-order): drow on DVE and
            # the main evacuation on GPSIMD run concurrently at sweep end.
            drow = npool.tile([1, 2, 512], F32, tag="drow", name="drow")
            nc.vector.tensor_copy(drow[:, :, :QB], o_ps[DH : DH + 1, :, :QB])
            oev = npool.tile([P, 2, 512], F32, tag="oev", name="oev")
            nc.scalar.copy(oev[0:DH, :, :QB], o_ps[0:DH, :, :QB])
            recip = npool.tile([1, 2, 512], F32, tag="recip", name="recip")
            nc.vector.reciprocal_approx_fast(out=recip[:, :, :QB], in_=drow[:, :, :QB])
            bcast = npool.tile([64, 2, 512], F32, tag="bcast", name="bcast")
            nc.gpsimd.partition_broadcast(bcast[:, :, :QB], recip[:, :, :QB])
            for a in range(2):
                nc.vector.tensor_tensor(
                    on[g, b][a * 64 : a * 64 + 64, :],
                    oev[0:DH, a, :QB],
                    bcast[:, a, :QB],
                    mybir.AluOpType.mult,
                )
            return
        # tail sweep: normalize in two query halves so the final out-proj
        # block starts while the second half is still normalizing; broadcast
        # via a PE rank-1 fp32 matmul (the PE is idle here, and it is much
        # lower latency than the GPSIMD partition_broadcast).
        HB = QB // 2
        oev = npool.tile([P, 2, 512], F32, tag="oev", name="oev")
        drow = npool.tile([1, 2, 512], F32, tag="drow", name="drow")
        recip = npool.tile([1, 2, 512], F32, tag="recip", name="recip")
        for hx in range(2):
            qs = slice(hx * HB, (hx + 1) * HB)
            nc.vector.tensor_copy(oev[0:DH, :, qs], o_ps[0:DH, :, qs])
            nc.vector.tensor_copy(drow[:, :, qs], o_ps[DH : DH + 1, :, qs])
            nc.vector.reciprocal_approx_fast(out=recip[:, :, qs], in_=drow[:, :, qs])
            for a in range(2):
                bc_ps = paQ.tile([P, 512], F32, tag="pa", name="bc_ps")
                nc.tensor.matmul(
                    bc_ps[0:DH, :HB],
                    lhsT=ones_f32[:],
                    rhs=recip[:, a, qs],
                    start=True,
                    stop=True,
                )
                nc.vector.tensor_tensor(
                    on[g, b][a * 64 : a * 64 + 64, qs],
                    oev[0:DH, a, qs],
                    bc_ps[0:DH, :HB],
                    mybir.AluOpType.mult,
                )

    NH = cfg.dim // 512
    ot_half = {}

    def out_proj_chunk(bb, t, nh, scalar_evac=False, pool=None):
        nt = bb * (QB // 128) + t
        if pool is None:
            ps = paQ.tile([P, 512], F32, tag="pa", name="pc")
        else:
            # tail: borrow idle psS/psO banks for a deeper psum pipeline
            ps = pool.tile([P, 2, 512], F32, tag=("s" if pool is psS else "o"), name="pc")[:, 0, :]
        for kc in range(M_SLABS):
            nc.tensor.matmul(
                ps[:],
                lhsT=on[kc, bb][:, ts(t, 128)],
                rhs=wo_sb[:, kc, ts(nh, 512)],
                start=(kc == 0),
                stop=(kc == M_SLABS - 1),
            )
        # both nh halves share one [128, 1024] tile; a single row-block DMA
        # fires with the second half (fewer DMAs -> less sem/teardown cost)
        if nh == 0:
            ot = copool.tile([P, 1024], MD, tag="ot", name="ot")
            ot_half[nt] = ot
        else:
            ot = ot_half.pop(nt)
        if scalar_evac:
            nc.scalar.copy(ot[:, ts(nh, 512)], ps[:])
        else:
            nc.vector.tensor_copy(ot[:, ts(nh, 512)], ps[:])
        if nh == NH - 1:
            nc.sync.dma_start(out[ts(nt, 128), :], ot[:])

    # ---- emission schedule (b-major: both head-pairs of a block, then the
    # next block; out-proj chunks for block b become fillers two sweeps on) ----
    emit_qk(wk_sb, kt, 0, 0)
    emitted.add(("k", 0, 0))
    emit_qk(wq_sb, qt, 0, 0)
    emitted.add(("q", 0, 0))

    for b in range(1, NQB):
        pend.append((("k", 0, b), partial(emit_qk, wk_sb, kt, 0, b)))
    for b in range(1, NQB):
        pend.append((("q", 0, b), partial(emit_qk, wq_sb, qt, 0, b)))
    pend.append((("q", 1, 0), partial(emit_qk, wq_sb, qt, 1, 0)))
    for b in range(NQB):
        pend.append((("k", 1, b), partial(emit_qk, wk_sb, kt, 1, b)))
    for b in range(1, NQB):
        pend.append((("q", 1, b), partial(emit_qk, wq_sb, qt, 1, b)))

    for g in range(PAIRS):
        for b in range(NQB):
            first = b == 0 and g == 0
            last = b == NQB - 1 and g == PAIRS - 1
            attention(b, g, with_v=first, slots=not first, tail=last)
            if g == PAIRS - 1 and b < NQB - 1:
                for t in range(QB // 128):
                    for nh in range(NH):
                        pend_small.append(
                            (("o", b, t, nh), partial(out_proj_chunk, b, t, nh))
                        )
    while pend:
        fill_one()
    while pend_small:
        fill_one(pend_small)
    # tail block: rotate through the now-idle psum pools for a deeper
    # pipeline, and alternate evac engines (ScalarE is idle after the last
    # exp)
    pools = [None, psS, psO]
    i = 0
    for t in range(QB // 128):
        for nh in range(NH):
            out_proj_chunk(NQB - 1, t, nh, scalar_evac=(nh == 0), pool=pools[i % 3])
            i += 1


def build_program(cfg, num_devices=N_CORES):
    nc = bacc.Bacc("TRN2", target_bir_lowering=False, debug=False, num_devices=num_devices)
    P = 128
    qkdt = F8 if cfg.qk_fp8 else cfg.mm_dt
    xT = nc.dram_tensor("xT", [P, cfg.kc, cfg.n], cfg.mm_dt, kind="ExternalInput").ap()
    xT8 = nc.dram_tensor("xT8", [P, cfg.kc, cfg.n], qkdt, kind="ExternalInput").ap()
    wq = nc.dram_tensor("wq", [P, cfg.kc, cfg.shard], qkdt, kind="ExternalInput").ap()
    wk = nc.dram_tensor("wk", [P, cfg.kc, cfg.shard], qkdt, kind="ExternalInput").ap()
    wv = nc.dram_tensor("wv", [P, cfg.kc, cfg.vw], cfg.mm_dt, kind="ExternalInput").ap()
    wo = nc.dram_tensor("wo", [P, cfg.shard // 128, cfg.dim], cfg.mm_dt, kind="ExternalInput").ap()
    out = nc.dram_tensor("out", [cfg.n, cfg.dim], cfg.mm_dt, kind="ExternalOutput").ap()
    with tile.TileContext(nc) as tc, ExitStack() as ctx:
        build_kernel(tc, ctx, cfg, xT, xT8, wq, wk, wv, wo, out)
    nc.compile()
    return nc


def shard_inputs(cfg, x, W_qkv, W_out, n_groups):
    """Build per-core input maps. Core c = (batch b, head-group g): c = b*n_groups + g."""
    import ml_dtypes

    f8 = ml_dtypes.float8_e4m3
    qk_dt = f8 if cfg.qk_fp8 else cfg.np_dt
    b_sz = x.shape[0]
    dim, hg, sh = cfg.dim, cfg.hg, cfg.shard
    xTs = []
    for b in range(b_sz):
        xt = np.ascontiguousarray(
            x[b].T.reshape(cfg.kc, 128, cfg.n).transpose(1, 0, 2)
        )
        xTs.append(xt)

    def wlayout(w):  # [dim, C] -> [128, kc, C]
        return np.ascontiguousarray(
            w.reshape(cfg.kc, 128, w.shape[1]).transpose(1, 0, 2)
        )

    in_maps = []
    for b in range(b_sz):
        for g in range(n_groups):
            wq = W_qkv[:, sh * g : sh * (g + 1)]
            wk = W_qkv[:, dim + sh * g : dim + sh * (g + 1)]
            wv_cols = W_qkv[:, 2 * dim + sh * g : 2 * dim + sh * (g + 1)]
            wv = np.zeros((dim, cfg.vw), np.float32)
            for h in range(hg):
                wv[:, h * (DH + 1) : h * (DH + 1) + DH] = wv_cols[:, h * DH : (h + 1) * DH]
            wo = W_out[sh * g : sh * (g + 1), :]
            wo_l = np.ascontiguousarray(
                wo.reshape(sh // 128, 128, dim).transpose(1, 0, 2)
            )
            in_maps.append(
                {
                    "xT": xTs[b].astype(cfg.np_dt),
                    "xT8": xTs[b].astype(qk_dt),
                    "wq": wlayout(wq).astype(qk_dt),
                    "wk": wlayout(wk).astype(qk_dt),
                    "wv": wlayout(wv).astype(cfg.np_dt),
                    "wo": wo_l.astype(cfg.np_dt),
                }
            )
    return in_maps


_NC_CACHE = {}


def kernel(x, W_qkv, W_out, b_out):
    x = np.asarray(x, np.float32)
    W_qkv = np.asarray(W_qkv, np.float32)
    W_out = np.asarray(W_out, np.float32)
    b_out = np.asarray(b_out, np.float32)
    cfg = FULL
    bsz = x.shape[0]
    n_groups = N_CORES // bsz

    if "nc" not in _NC_CACHE:
        _NC_CACHE["nc"] = build_program(cfg)
    nc = _NC_CACHE["nc"]

    in_maps = shard_inputs(cfg, x, W_qkv, W_out, n_groups)
    res = run_bass_kernel_spmd(nc, in_maps, list(range(N_CORES)))

    out = np.zeros((bsz, cfg.n, cfg.dim), np.float32)
    for b in range(bsz):
        for g in range(n_groups):
            out[b] += res.results[b * n_groups + g]["out"].astype(np.float32)
        out[b] += b_out
    return out


# revision 36
# speedup vs baseline: 3.4565x; 1.1306x over previous
"""Multi-head attention (b=2, n=2048, dim=1024, h=16, fp32) on 8 TRN2 NeuronCores.

Sharding: 2 batches x 4 head-groups (4 heads each). Each core computes, for its
batch element and 4 heads: QKV projection, softmax attention, and a partial
output projection (W_out rows of its heads). Host sums the 4 partials per batch
and adds the bias.

Device layout choices (per core):
  - x arrives pre-transposed (host) as xT [128, 8, 2048] fp16 plus an fp8e4
    copy xT8; W_q/W_k arrive fp8e4, W_v/W_o fp16.
  - Q^T/K^T computed as [128, 2048] per head-pair via fp8 DoubleRow matmuls
    (contraction 256 per pass -> half the passes of fp16); V kept fp16 for
    precision (fp8 on the V path costs ~3.6% rel err, over budget).
  - S^T = K @ Q^T per head via row-tiled (K=64) fp16 matmul pairs; softmax exp
    on ScalarE directly PSUM->SBUF with scale=dim^-0.5 folded in (no max
    subtraction needed: |scores*scale| < ~0.5).
  - V is augmented with a ones column per head ([V_h | 1]) so the PV matmul's
    65th output row accumulates the softmax denominator for free.
  - Normalization: reciprocal_approx_fast (DVE) + partition_broadcast (GPSIMD)
    + one tensor_tensor multiply; the last sweep instead broadcasts via a PE
    rank-1 matmul (ones x recip) to cut tail latency.
  - Schedule: b-major sweeps with just-in-time projection fillers and
    out-projection chunks interleaved into later sweeps; kc-granular DMAs so
    the first matmuls start ~2us in; dummy warmup matmuls ramp the PE p-state
    during the initial DMA window.
"""

import os
import numpy as np
from contextlib import ExitStack
from collections import deque
from functools import partial

import concourse.bass as bass
import concourse.mybir as mybir
import concourse.tile as tile
from concourse import bacc
from concourse.bass import ts
from concourse.bass_utils import run_bass_kernel_spmd

F32 = mybir.dt.float32
F16 = mybir.dt.float16
F8 = mybir.dt.float8e4
DRMODE = mybir.MatmulPerfMode.DoubleRow

N_CORES = 8
HEADS = 16
DH = 64  # head dim


class Cfg:
    def __init__(self, n, dim, hg):
        self.n = n                    # sequence length (per core)
        self.dim = dim                # model dim
        self.hg = hg                  # heads per core
        self.kc = dim // 128          # dim chunks of 128
        self.nqb = max(1, n // 512)   # query blocks of 512
        self.qb = min(n, 512)
        self.nkc = n // 128           # key chunks of 128
        self.pairs = hg // 2
        self.shard = hg * DH          # qkv shard columns per section
        self.vw = hg * (DH + 1)       # V columns incl per-head ones col
        self.mm_dt = F16
        self.np_dt = np.float16
        self.qk_fp8 = os.environ.get("ATTN_QK_FP8", "1") == "1"


FULL = Cfg(2048, 1024, 4)


def build_kernel(tc, ctx, cfg, xT, xT8, wq, wk, wv, wo, out):
    nc = tc.nc
    P = 128
    KC, NQB, QB, NKC, PAIRS = cfg.kc, cfg.nqb, cfg.qb, cfg.nkc, cfg.pairs
    MD = cfg.mm_dt
    SCALE = cfg.dim ** -0.5
    M_SLABS = cfg.shard // 128  # = PAIRS
    QKDT = F8 if cfg.qk_fp8 else MD

    wpool = ctx.enter_context(tc.tile_pool(name="w", bufs=1))
    wq_sb = wpool.tile([P, KC, cfg.shard], QKDT, tag="wq", name="wq_sb")
    wk_sb = wpool.tile([P, KC, cfg.shard], QKDT, tag="wk", name="wk_sb")
    wv_sb = wpool.tile([P, KC, cfg.vw], MD, tag="wv", name="wv_sb")
    wo_sb = wpool.tile([P, M_SLABS, cfg.dim], MD, tag="wo", name="wo_sb")

    per = ctx.enter_context(tc.tile_pool(name="per", bufs=1))
    qt = {}  # (pair, nqb) -> [128, QB]
    kt = {}
    vt = {}  # nt -> [128, vw]
    on = {}  # (slab, nqb) -> [128, QB]  normalized O^T for out-proj lhsT
    for g in range(PAIRS):
        for b in range(NQB):
            qt[g, b] = per.tile([P, QB], MD, tag=f"qt{g}_{b}", name=f"qt{g}_{b}")
            kt[g, b] = per.tile([P, QB], MD, tag=f"kt{g}_{b}", name=f"kt{g}_{b}")
            on[g, b] = per.tile([P, QB], MD, tag=f"on{g}_{b}", name=f"on{g}_{b}")
    for t in range(NKC):
        vt[t] = per.tile([P, cfg.vw], MD, tag=f"v{t}", name=f"v{t}")

    xpool = ctx.enter_context(tc.tile_pool(name="x", bufs=1))
    paQ = ctx.enter_context(tc.tile_pool(name="paQ", bufs=2, space="PSUM"))
    psS = ctx.enter_context(tc.tile_pool(name="psS", bufs=2, space="PSUM"))
    psO = ctx.enter_context(tc.tile_pool(name="psO", bufs=1, space="PSUM"))
    epool = ctx.enter_context(tc.tile_pool(name="e", bufs=10))
    npool = ctx.enter_context(tc.tile_pool(name="nrm", bufs=3))
    copool = ctx.enter_context(tc.tile_pool(name="co", bufs=6))

    xts = {}
    x8s = {}
    for b in range(NQB):
        xts[b] = xpool.tile([P, KC, QB], MD, tag=f"xt{b}", name=f"xt{b}")
        if cfg.qk_fp8:
            x8s[b] = xpool.tile([P, KC, QB], F8, tag=f"x8{b}", name=f"x8{b}")
        else:
            x8s[b] = xts[b]

    # PE warmup: ~40 tiny matmuls on a zeroed tile ramp the tensor engine's
    # p-state while the first DMAs land.
    wt = xpool.tile([P, 64], MD, tag="warm", name="warm")
    nc.vector.memset(wt[:], 0.0)
    for i in range(16):
        ps = paQ.tile([P, 512], F32, tag="pa", name="warm_ps")
        nc.tensor.matmul(ps[0:64, 0:64], lhsT=wt[:], rhs=wt[:], start=True, stop=True)

    # DMA order follows first-use order: K weights + x8 block 0 (first K^T
    # emit), Q weights (Q^T), then x block 0 + V weights (V emits from c=0),
    # then the remaining blocks.
    h = KC // 2
    x80 = xT8 if cfg.qk_fp8 else xT
    nc.sync.dma_start(wk_sb[:, :h], wk[:, :h])
    nc.sync.dma_start(x8s[0][:, :h], x80[:, :h, ts(0, QB)])
    nc.sync.dma_start(wk_sb[:, h:], wk[:, h:])
    nc.sync.dma_start(x8s[0][:, h:], x80[:, h:, ts(0, QB)])
    nc.sync.dma_start(wq_sb[:], wq[:])
    if cfg.qk_fp8:
        nc.sync.dma_start(xts[0][:, :h], xT[:, :h, ts(0, QB)])
        nc.sync.dma_start(xts[0][:, h:], xT[:, h:, ts(0, QB)])
    nc.sync.dma_start(wv_sb[:, :h], wv[:, :h])
    nc.sync.dma_start(wv_sb[:, h:], wv[:, h:])
    for b in range(1, NQB):
        nc.sync.dma_start(xts[b][:, :h], xT[:, :h, ts(b, QB)])
        nc.sync.dma_start(xts[b][:, h:], xT[:, h:, ts(b, QB)])
        if cfg.qk_fp8:
            nc.sync.dma_start(x8s[b][:], xT8[:, :, ts(b, QB)])
    nc.sync.dma_start(wo_sb[:], wo[:])

    def emit_qk(w_sb, dst, g, b):
        # Q^T / K^T slab for head-pair g, query block b.
        ps = paQ.tile([P, 512], F32, tag="pa", name="pa")
        if cfg.qk_fp8:
            for j in range(KC // 2):
                nc.tensor.matmul(
                    ps[:, :QB],
                    lhsT=w_sb[:, 2 * j : 2 * j + 2, ts(g, 128)],
                    rhs=x8s[b][:, 2 * j : 2 * j + 2, :],
                    start=(j == 0),
                    stop=(j == KC // 2 - 1),
                    perf_mode=DRMODE,
                )
        else:
            for kc in range(KC):
                nc.tensor.matmul(
                    ps[:, :QB],
                    lhsT=w_sb[:, kc, ts(g, 128)],
                    rhs=x8s[b][:, kc, :],
                    start=(kc == 0),
                    stop=(kc == KC - 1),
                )
        nc.vector.tensor_copy(dst[g, b][:], ps[:, :QB])

    def emit_v(nt):
        vb, t = divmod(nt, QB // 128)
        ps = paQ.tile([P, 512], F32, tag="pa", name="pa")
        for kc in range(KC):
            nc.tensor.matmul(
                ps[:, : cfg.vw],
                lhsT=xts[vb][:, kc, ts(t, 128)],
                rhs=wv_sb[:, kc, :],
                start=(kc == 0),
                stop=(kc == KC - 1),
            )
        nc.vector.tensor_copy(vt[nt][:], ps[:, : cfg.vw])
        v4 = vt[nt][:].rearrange("p (h e) -> p h e", e=DH + 1)
        nc.vector.memset(v4[:, :, DH : DH + 1], 1.0)

    # ---- filler machinery: pending emissions pulled into sweeps ----
    pend = deque()        # big fillers: Q^T/K^T slab emissions (~1.8us each)
    pend_small = deque()  # small fillers: out-proj chunks (~0.5us each)
    emitted = set()

    def fill_one(q=None):
        key, fn = (q or pend).popleft()
        fn()
        emitted.add(key)

    def require(*keys):
        while pend and any(k not in emitted for k in keys):
            fill_one()

    ones_f32 = npool.tile([1, DH], F32, tag="ones", name="ones_f32")
    nc.vector.memset(ones_f32[:], 1.0)

    def attention(b, g, with_v=False, slots=True, tail=False):
        require(("q", g, b), ("k", g, 0))
        o_ps = psO.tile([P, 2, 512], F32, tag="o", name="o_ps")
        e_ts = {}
        pvq = deque()
        npv = [0]

        def emit_pv(c):
            v4 = vt[c][:].rearrange("p (h e) -> p h e", e=DH + 1)
            for a in range(2):
                h = 2 * g + a
                nc.tensor.matmul(
                    o_ps[0 : DH + 1, a, :QB],
                    lhsT=v4[:, h, :],
                    rhs=e_ts[c][:, a, :QB],
                    start=(npv[0] == 0),
                    stop=(npv[0] == NKC - 1),
                )
            npv[0] += 1

        for c in range(NKC):
            cb = c * 128 // QB
            if c == 4 * cb and cb > 0:
                require(("k", g, cb))
            s_ps = psS.tile([P, 2, 512], F32, tag="s", name="s_ps")
            for a in range(2):
                lo = a * 64
                nc.tensor.matmul(
                    s_ps[:, a, :QB],
                    lhsT=kt[g, cb][lo : lo + 64, ts(c % (QB // 128), 128)],
                    rhs=qt[g, b][lo : lo + 64, :],
                    start=True,
                    stop=True,
                )
            if with_v:
                emit_v(c)
            if slots:
                # small (out-proj) fillers slot in at any odd chunk; big
                # projection fillers only once per sweep to avoid bunching
                if pend_small and c % 2 == 1:
                    fill_one(pend_small)
                elif pend and c == 9:
                    fill_one()
            e_t = epool.tile([P, 2, 512], MD, tag="e", name="e_t")
            e_ts[c] = e_t
            nc.scalar.activation(
                e_t[:, :, :QB],
                s_ps[:, :, :QB],
                mybir.ActivationFunctionType.Exp,
                scale=SCALE,
            )
            pvq.append(c)
            while pvq:
                emit_pv(pvq.popleft())
        while pvq:
            emit_pv(pvq.popleft())
        # normalize; stage the denom row at partition 0 (the custom DVE
        # reciprocal misreads inputs at a nonzero base partition)
        if not tail:
            # o_ps must drain before the next sweep's first PV matmul (psO is
            # single-buffered and the PE queue is in-order): drow on DVE and
            # the main evacuation on GPSIMD run concurrently at sweep end.
            drow = npool.tile([1, 2, 512], F32, tag="drow", name="drow")
            nc.vector.tensor_copy(drow[:, :, :QB], o_ps[DH : DH + 1, :, :QB])
            oev = npool.tile([P, 2, 512], F32, tag="oev", name="oev")
            nc.scalar.copy(oev[0:DH, :, :QB], o_ps[0:DH, :, :QB])
            recip = npool.tile([1, 2, 512], F32, tag="recip", name="recip")
            nc.vector.reciprocal_approx_fast(out=recip[:, :, :QB], in_=drow[:, :, :QB])
            bcast = npool.tile([64, 2, 512], F32, tag="bcast", name="bcast")
            nc.gpsimd.partition_broadcast(bcast[:, :, :QB], recip[:, :, :QB])
            for a in range(2):
                nc.vector.tensor_tensor(
                    on[g, b][a * 64 : a * 64 + 64, :],
                    oev[0:DH, a, :QB],
                    bcast[:, a, :QB],
                    mybir.AluOpType.mult,
                )
            return
        # tail sweep: normalize in two query halves so the final out-proj
        # block starts while the second half is still normalizing; broadcast
        # via a PE rank-1 fp32 matmul (the PE is idle here, and it is much
        # lower latency than the GPSIMD partition_broadcast).
        HB = QB // 2
        oev = npool.tile([P, 2, 512], F32, tag="oev", name="oev")
        drow = npool.tile([1, 2, 512], F32, tag="drow", name="drow")
        recip = npool.tile([1, 2, 512], F32, tag="recip", name="recip")
        for hx in range(2):
            qs = slice(hx * HB, (hx + 1) * HB)
            nc.vector.tensor_copy(oev[0:DH, :, qs], o_ps[0:DH, :, qs])
            nc.vector.tensor_copy(drow[:, :, qs], o_ps[DH : DH + 1, :, qs])
            nc.vector.reciprocal_approx_fast(out=recip[:, :, qs], in_=drow[:, :, qs])
            for a in range(2):
                bc_ps = paQ.tile([P, 512], F32, tag="pa", name="bc_ps")
                nc.tensor.matmul(
                    bc_ps[0:DH, :HB],
                    lhsT=ones_f32[:],
                    rhs=recip[:, a, qs],
                    start=True,
                    stop=True,
                )
                nc.vector.tensor_tensor(
                    on[g, b][a * 64 : a * 64 + 64, qs],
                    oev[0:DH, a, qs],
                    bc_ps[0:DH, :HB],
                    mybir.AluOpType.mult,
                )

    NH = cfg.dim // 512
    ot_half = {}

    def out_proj_chunk(bb, t, nh, scalar_evac=False, pool=None):
        nt = bb * (QB // 128) + t
        if pool is None:
            ps = paQ.tile([P, 512], F32, tag="pa", name="pc")
        else:
            # tail: borrow idle psS/psO banks for a deeper psum pipeline
            ps = pool.tile([P, 2, 512], F32, tag=("s" if pool is psS else "o"), name="pc")[:, 0, :]
        for kc in range(M_SLABS):
            nc.tensor.matmul(
                ps[:],
                lhsT=on[kc, bb][:, ts(t, 128)],
                rhs=wo_sb[:, kc, ts(nh, 512)],
                start=(kc == 0),
                stop=(kc == M_SLABS - 1),
            )
        # both nh halves share one [128, 1024] tile; a single row-block DMA
        # fires with the second half (fewer DMAs -> less sem/teardown cost)
        if nh == 0:
            ot = copool.tile([P, 1024], MD, tag="ot", name="ot")
            ot_half[nt] = ot
        else:
            ot = ot_half.pop(nt)
        if scalar_evac:
            nc.scalar.copy(ot[:, ts(nh, 512)], ps[:])
        else:
            nc.vector.tensor_copy(ot[:, ts(nh, 512)], ps[:])
        if nh == NH - 1:
            nc.sync.dma_start(out[ts(nt, 128), :], ot[:])

    # ---- emission schedule (b-major: both head-pairs of a block, then the
    # next block; out-proj chunks for block b become fillers two sweeps on) ----
    emit_qk(wk_sb, kt, 0, 0)
    emitted.add(("k", 0, 0))
    emit_qk(wq_sb, qt, 0, 0)
    emitted.add(("q", 0, 0))

    for b in range(1, NQB):
        pend.append((("k", 0, b), partial(emit_qk, wk_sb, kt, 0, b)))
    for b in range(1, NQB):
        pend.append((("q", 0, b), partial(emit_qk, wq_sb, qt, 0, b)))
    pend.append((("q", 1, 0), partial(emit_qk, wq_sb, qt, 1, 0)))
    for b in range(NQB):
        pend.append((("k", 1, b), partial(emit_qk, wk_sb, kt, 1, b)))
    for b in range(1, NQB):
        pend.append((("q", 1, b), partial(emit_qk, wq_sb, qt, 1, b)))

    for g in range(PAIRS):
        for b in range(NQB):
            first = b == 0 and g == 0
            last = b == NQB - 1 and g == PAIRS - 1
            attention(b, g, with_v=first, slots=not first, tail=last)
            if g == PAIRS - 1 and b < NQB - 1:
                for t in range(QB // 128):
                    for nh in range(NH):
                        pend_small.append(
                            (("o", b, t, nh), partial(out_proj_chunk, b, t, nh))
                        )
    while pend:
        fill_one()
    while pend_small:
        fill_one(pend_small)
    # tail block: rotate through the now-idle psum pools for a deeper
    # pipeline, and alternate evac engines (ScalarE is idle after the last
    # exp)
    pools = [None, psS, psO]
    i = 0
    for t in range(QB // 128):
        for nh in range(NH):
            out_proj_chunk(NQB - 1, t, nh, scalar_evac=(nh == 0), pool=pools[i % 3])
            i += 1


def build_program(cfg, num_devices=N_CORES):
    nc = bacc.Bacc("TRN2", target_bir_lowering=False, debug=False, num_devices=num_devices)
    P = 128
    qkdt = F8 if cfg.qk_fp8 else cfg.mm_dt
    xT = nc.dram_tensor("xT", [P, cfg.kc, cfg.n], cfg.mm_dt, kind="ExternalInput").ap()
    xT8 = nc.dram_tensor("xT8", [P, cfg.kc, cfg.n], qkdt, kind="ExternalInput").ap()
    wq = nc.dram_tensor("wq", [P, cfg.kc, cfg.shard], qkdt, kind="ExternalInput").ap()
    wk = nc.dram_tensor("wk", [P, cfg.kc, cfg.shard], qkdt, kind="ExternalInput").ap()
    wv = nc.dram_tensor("wv", [P, cfg.kc, cfg.vw], cfg.mm_dt, kind="ExternalInput").ap()
    wo = nc.dram_tensor("wo", [P, cfg.shard // 128, cfg.dim], cfg.mm_dt, kind="ExternalInput").ap()
    out = nc.dram_tensor("out", [cfg.n, cfg.dim], cfg.mm_dt, kind="ExternalOutput").ap()
    with tile.TileContext(nc) as tc, ExitStack() as ctx:
        build_kernel(tc, ctx, cfg, xT, xT8, wq, wk, wv, wo, out)
    nc.compile()
    return nc


def shard_inputs(cfg, x, W_qkv, W_out, n_groups):
    """Build per-core input maps. Core c = (batch b, head-group g): c = b*n_groups + g."""
    import ml_dtypes

    f8 = ml_dtypes.float8_e4m3
    qk_dt = f8 if cfg.qk_fp8 else cfg.np_dt
    b_sz = x.shape[0]
    dim, hg, sh = cfg.dim, cfg.hg, cfg.shard
    xTs = []
    for b in range(b_sz):
        xt = np.ascontiguousarray(
            x[b].T.reshape(cfg.kc, 128, cfg.n).transpose(1, 0, 2)
        )
        xTs.append(xt)

    def wlayout(w):  # [dim, C] -> [128, kc, C]
        return np.ascontiguousarray(
            w.reshape(cfg.kc, 128, w.shape[1]).transpose(1, 0, 2)
        )

    in_maps = []
    for b in range(b_sz):
        for g in range(n_groups):
            wq = W_qkv[:, sh * g : sh * (g + 1)]
            wk = W_qkv[:, dim + sh * g : dim + sh * (g + 1)]
            wv_cols = W_qkv[:, 2 * dim + sh * g : 2 * dim + sh * (g + 1)]
            wv = np.zeros((dim, cfg.vw), np.float32)
            for h in range(hg):
                wv[:, h * (DH + 1) : h * (DH + 1) + DH] = wv_cols[:, h * DH : (h + 1) * DH]
            wo = W_out[sh * g : sh * (g + 1), :]
            wo_l = np.ascontiguousarray(
                wo.reshape(sh // 128, 128, dim).transpose(1, 0, 2)
            )
            in_maps.append(
                {
                    "xT": xTs[b].astype(cfg.np_dt),
                    "xT8": xTs[b].astype(qk_dt),
                    "wq": wlayout(wq).astype(qk_dt),
                    "wk": wlayout(wk).astype(qk_dt),
                    "wv": wlayout(wv).astype(cfg.np_dt),
                    "wo": wo_l.astype(cfg.np_dt),
                }
            )
    return in_maps


_NC_CACHE = {}


def kernel(x, W_qkv, W_out, b_out):
    x = np.asarray(x, np.float32)
    W_qkv = np.asarray(W_qkv, np.float32)
    W_out = np.asarray(W_out, np.float32)
    b_out = np.asarray(b_out, np.float32)
    cfg = FULL
    bsz = x.shape[0]
    n_groups = N_CORES // bsz

    if "nc" not in _NC_CACHE:
        _NC_CACHE["nc"] = build_program(cfg)
    nc = _NC_CACHE["nc"]

    in_maps = shard_inputs(cfg, x, W_qkv, W_out, n_groups)
    res = run_bass_kernel_spmd(nc, in_maps, list(range(N_CORES)))

    out = np.zeros((bsz, cfg.n, cfg.dim), np.float32)
    for b in range(bsz):
        for g in range(n_groups):
            out[b] += res.results[b * n_groups + g]["out"].astype(np.float32)
        out[b] += b_out
    return out


# revision 39
# speedup vs baseline: 3.4925x; 1.0104x over previous
"""Multi-head attention (b=2, n=2048, dim=1024, h=16, fp32) on 8 TRN2 NeuronCores.

Sharding: 2 batches x 4 head-groups (4 heads each). Each core computes, for its
batch element and 4 heads: QKV projection, softmax attention, and a partial
output projection (W_out rows of its heads). Host sums the 4 partials per batch
and adds the bias.

Device layout choices (per core):
  - x arrives pre-transposed (host) as xT [128, 8, 2048] fp16 plus an fp8e4
    copy xT8; W_q/W_k arrive fp8e4, W_v/W_o fp16.
  - Q^T/K^T computed as [128, 2048] per head-pair via fp8 DoubleRow matmuls
    (contraction 256 per pass -> half the passes of fp16); V kept fp16 for
    precision (fp8 on the V path costs ~3.6% rel err, over budget).
  - S^T = K @ Q^T per head via row-tiled (K=64) fp16 matmul pairs; softmax exp
    on ScalarE directly PSUM->SBUF with scale=dim^-0.5 folded in (no max
    subtraction needed: |scores*scale| < ~0.5).
  - V is augmented with a ones column per head ([V_h | 1]) so the PV matmul's
    65th output row accumulates the softmax denominator for free.
  - Normalization: reciprocal_approx_fast (DVE) + partition_broadcast (GPSIMD)
    + one tensor_tensor multiply; the last sweep instead broadcasts via a PE
    rank-1 matmul (ones x recip) to cut tail latency.
  - Schedule: b-major sweeps with just-in-time projection fillers and
    out-projection chunks interleaved into later sweeps; kc-granular DMAs so
    the first matmuls start ~2us in; dummy warmup matmuls ramp the PE p-state
    during the initial DMA window.
"""

import os
import numpy as np
from contextlib import ExitStack
from collections import deque
from functools import partial

import concourse.bass as bass
import concourse.mybir as mybir
import concourse.tile as tile
from concourse import bacc
from concourse.bass import ts
from concourse.bass_utils import run_bass_kernel_spmd

F32 = mybir.dt.float32
F16 = mybir.dt.float16
F8 = mybir.dt.float8e4
DRMODE = mybir.MatmulPerfMode.DoubleRow

N_CORES = 8
HEADS = 16
DH = 64  # head dim


class Cfg:
    def __init__(self, n, dim, hg):
        self.n = n                    # sequence length (per core)
        self.dim = dim                # model dim
        self.hg = hg                  # heads per core
        self.kc = dim // 128          # dim chunks of 128
        self.nqb = max(1, n // 512)   # query blocks of 512
        self.qb = min(n, 512)
        self.nkc = n // 128           # key chunks of 128
        self.pairs = hg // 2
        self.shard = hg * DH          # qkv shard columns per section
        self.vw = hg * (DH + 1)       # V columns incl per-head ones col
        self.mm_dt = F16
        self.np_dt = np.float16
        self.qk_fp8 = os.environ.get("ATTN_QK_FP8", "1") == "1"


FULL = Cfg(2048, 1024, 4)


def build_kernel(tc, ctx, cfg, xT, xT8, wq, wk, wv, wo, out):
    nc = tc.nc
    P = 128
    KC, NQB, QB, NKC, PAIRS = cfg.kc, cfg.nqb, cfg.qb, cfg.nkc, cfg.pairs
    MD = cfg.mm_dt
    SCALE = cfg.dim ** -0.5
    M_SLABS = cfg.shard // 128  # = PAIRS
    QKDT = F8 if cfg.qk_fp8 else MD

    wpool = ctx.enter_context(tc.tile_pool(name="w", bufs=1))
    wq_sb = wpool.tile([P, KC, cfg.shard], QKDT, tag="wq", name="wq_sb")
    wk_sb = wpool.tile([P, KC, cfg.shard], QKDT, tag="wk", name="wk_sb")
    wv_sb = wpool.tile([P, KC, cfg.vw], MD, tag="wv", name="wv_sb")
    wo_sb = wpool.tile([P, M_SLABS, cfg.dim], MD, tag="wo", name="wo_sb")

    per = ctx.enter_context(tc.tile_pool(name="per", bufs=1))
    qt = {}  # (pair, nqb) -> [128, QB]
    kt = {}
    vt = {}  # nt -> [128, vw]
    on = {}  # (slab, nqb) -> [128, QB]  normalized O^T for out-proj lhsT
    for g in range(PAIRS):
        for b in range(NQB):
            qt[g, b] = per.tile([P, QB], MD, tag=f"qt{g}_{b}", name=f"qt{g}_{b}")
            kt[g, b] = per.tile([P, QB], MD, tag=f"kt{g}_{b}", name=f"kt{g}_{b}")
            on[g, b] = per.tile([P, QB], MD, tag=f"on{g}_{b}", name=f"on{g}_{b}")
    for t in range(NKC):
        vt[t] = per.tile([P, cfg.vw], MD, tag=f"v{t}", name=f"v{t}")

    xpool = ctx.enter_context(tc.tile_pool(name="x", bufs=1))
    paQ = ctx.enter_context(tc.tile_pool(name="paQ", bufs=2, space="PSUM"))
    psS = ctx.enter_context(tc.tile_pool(name="psS", bufs=2, space="PSUM"))
    psO = ctx.enter_context(tc.tile_pool(name="psO", bufs=1, space="PSUM"))
    epool = ctx.enter_context(tc.tile_pool(name="e", bufs=10))
    npool = ctx.enter_context(tc.tile_pool(name="nrm", bufs=3))
    copool = ctx.enter_context(tc.tile_pool(name="co", bufs=6))

    xts = {}
    x8s = {}
    for b in range(NQB):
        xts[b] = xpool.tile([P, KC, QB], MD, tag=f"xt{b}", name=f"xt{b}")
        if cfg.qk_fp8:
            x8s[b] = xpool.tile([P, KC, QB], F8, tag=f"x8{b}", name=f"x8{b}")
        else:
            x8s[b] = xts[b]

    # PE warmup: ~40 tiny matmuls on a zeroed tile ramp the tensor engine's
    # p-state while the first DMAs land.
    wt = xpool.tile([P, 64], MD, tag="warm", name="warm")
    nc.vector.memset(wt[:], 0.0)
    for i in range(26):
        ps = paQ.tile([P, 512], F32, tag="pa", name="warm_ps")
        nc.tensor.matmul(ps[0:64, 0:64], lhsT=wt[:], rhs=wt[:], start=True, stop=True)

    # DMA order follows first-use order: K weights + x8 block 0 (first K^T
    # emit), Q weights (Q^T), then x block 0 + V weights (V emits from c=0),
    # then the remaining blocks.
    h = KC // 2
    x80 = xT8 if cfg.qk_fp8 else xT
    nc.sync.dma_start(wk_sb[:, :h], wk[:, :h])
    nc.sync.dma_start(x8s[0][:, :h], x80[:, :h, ts(0, QB)])
    nc.sync.dma_start(wk_sb[:, h:], wk[:, h:])
    nc.sync.dma_start(x8s[0][:, h:], x80[:, h:, ts(0, QB)])
    nc.sync.dma_start(wq_sb[:], wq[:])
    if cfg.qk_fp8:
        nc.sync.dma_start(xts[0][:, :h], xT[:, :h, ts(0, QB)])
        nc.sync.dma_start(xts[0][:, h:], xT[:, h:, ts(0, QB)])
    nc.sync.dma_start(wv_sb[:, :h], wv[:, :h])
    nc.sync.dma_start(wv_sb[:, h:], wv[:, h:])
    for b in range(1, NQB):
        nc.sync.dma_start(xts[b][:, :h], xT[:, :h, ts(b, QB)])
        nc.sync.dma_start(xts[b][:, h:], xT[:, h:, ts(b, QB)])
        if cfg.qk_fp8:
            nc.sync.dma_start(x8s[b][:], xT8[:, :, ts(b, QB)])
    nc.sync.dma_start(wo_sb[:], wo[:])

    def emit_qk(w_sb, dst, g, b):
        # Q^T / K^T slab for head-pair g, query block b.
        ps = paQ.tile([P, 512], F32, tag="pa", name="pa")
        if cfg.qk_fp8:
            for j in range(KC // 2):
                nc.tensor.matmul(
                    ps[:, :QB],
                    lhsT=w_sb[:, 2 * j : 2 * j + 2, ts(g, 128)],
                    rhs=x8s[b][:, 2 * j : 2 * j + 2, :],
                    start=(j == 0),
                    stop=(j == KC // 2 - 1),
                    perf_mode=DRMODE,
                )
        else:
            for kc in range(KC):
                nc.tensor.matmul(
                    ps[:, :QB],
                    lhsT=w_sb[:, kc, ts(g, 128)],
                    rhs=x8s[b][:, kc, :],
                    start=(kc == 0),
                    stop=(kc == KC - 1),
                )
        nc.vector.tensor_copy(dst[g, b][:], ps[:, :QB])

    def emit_v(nt):
        vb, t = divmod(nt, QB // 128)
        ps = paQ.tile([P, 512], F32, tag="pa", name="pa")
        for kc in range(KC):
            nc.tensor.matmul(
                ps[:, : cfg.vw],
                lhsT=xts[vb][:, kc, ts(t, 128)],
                rhs=wv_sb[:, kc, :],
                start=(kc == 0),
                stop=(kc == KC - 1),
            )
        nc.vector.tensor_copy(vt[nt][:], ps[:, : cfg.vw])
        v4 = vt[nt][:].rearrange("p (h e) -> p h e", e=DH + 1)
        nc.vector.memset(v4[:, :, DH : DH + 1], 1.0)

    # ---- filler machinery: pending emissions pulled into sweeps ----
    pend = deque()        # big fillers: Q^T/K^T slab emissions (~1.8us each)
    pend_small = deque()  # small fillers: out-proj chunks (~0.5us each)
    emitted = set()

    def fill_one(q=None):
        key, fn = (q or pend).popleft()
        fn()
        emitted.add(key)

    def require(*keys):
        while pend and any(k not in emitted for k in keys):
            fill_one()

    ones_f32 = npool.tile([1, DH], F32, tag="ones", name="ones_f32")
    nc.vector.memset(ones_f32[:], 1.0)

    def attention(b, g, with_v=False, slots=True, tail=False):
        require(("q", g, b), ("k", g, 0))
        o_ps = psO.tile([P, 2, 512], F32, tag="o", name="o_ps")
        e_ts = {}
        pvq = deque()
        npv = [0]

        def emit_pv(c):
            v4 = vt[c][:].rearrange("p (h e) -> p h e", e=DH + 1)
            for a in range(2):
                h = 2 * g + a
                nc.tensor.matmul(
                    o_ps[0 : DH + 1, a, :QB],
                    lhsT=v4[:, h, :],
                    rhs=e_ts[c][:, a, :QB],
                    start=(npv[0] == 0),
                    stop=(npv[0] == NKC - 1),
                )
            npv[0] += 1

        for c in range(NKC):
            cb = c * 128 // QB
            if c == 4 * cb and cb > 0:
                require(("k", g, cb))
            s_ps = psS.tile([P, 2, 512], F32, tag="s", name="s_ps")
            for a in range(2):
                lo = a * 64
                nc.tensor.matmul(
                    s_ps[:, a, :QB],
                    lhsT=kt[g, cb][lo : lo + 64, ts(c % (QB // 128), 128)],
                    rhs=qt[g, b][lo : lo + 64, :],
                    start=True,
                    stop=True,
                )
            if with_v:
                emit_v(c)
            if slots:
                # small (out-proj) fillers slot in at any odd chunk; big
                # projection fillers only once per sweep to avoid bunching
                if pend_small and c % 2 == 1:
                    fill_one(pend_small)
                elif pend and c == 9:
                    fill_one()
            e_t = epool.tile([P, 2, 512], MD, tag="e", name="e_t")
            e_ts[c] = e_t
            nc.scalar.activation(
                e_t[:, :, :QB],
                s_ps[:, :, :QB],
                mybir.ActivationFunctionType.Exp,
                scale=SCALE,
            )
            pvq.append(c)
            while pvq:
                emit_pv(pvq.popleft())
        while pvq:
            emit_pv(pvq.popleft())
        # normalize; stage the denom row at partition 0 (the custom DVE
        # reciprocal misreads inputs at a nonzero base partition)
        if not tail:
            # o_ps must drain before the next sweep's first PV matmul (psO is
            # single-buffered and the PE queue is in-order): drow on DVE and
            # the main evacuation on GPSIMD run concurrently at sweep end.
            drow = npool.tile([1, 2, 512], F32, tag="drow", name="drow")
            nc.vector.tensor_copy(drow[:, :, :QB], o_ps[DH : DH + 1, :, :QB])
            oev = npool.tile([P, 2, 512], F32, tag="oev", name="oev")
            nc.scalar.copy(oev[0:DH, :, :QB], o_ps[0:DH, :, :QB])
            recip = npool.tile([1, 2, 512], F32, tag="recip", name="recip")
            nc.vector.reciprocal_approx_fast(out=recip[:, :, :QB], in_=drow[:, :, :QB])
            bcast = npool.tile([64, 2, 512], F32, tag="bcast", name="bcast")
            nc.gpsimd.partition_broadcast(bcast[:, :, :QB], recip[:, :, :QB])
            for a in range(2):
                nc.vector.tensor_tensor(
                    on[g, b][a * 64 : a * 64 + 64, :],
                    oev[0:DH, a, :QB],
                    bcast[:, a, :QB],
                    mybir.AluOpType.mult,
                )
            return
        # tail sweep: normalization is handled by the scheduler (interleaved
        # with the final out-proj block), so just hand back the accumulator
        return o_ps

    tail_tiles = {}

    def tail_norm(o_ps, g, b, hx):
        # normalize one query half; broadcast via a PE rank-1 fp32 matmul
        # (the PE is idle in the tail and it beats the GPSIMD
        # partition_broadcast on latency)
        HB = QB // 2
        if not tail_tiles:
            tail_tiles["oev"] = npool.tile([P, 2, 512], F32, tag="oev", name="oev")
            tail_tiles["drow"] = npool.tile([1, 2, 512], F32, tag="drow", name="drow")
            tail_tiles["recip"] = npool.tile([1, 2, 512], F32, tag="recip", name="recip")
        oev, drow, recip = tail_tiles["oev"], tail_tiles["drow"], tail_tiles["recip"]
        qs = slice(hx * HB, (hx + 1) * HB)
        nc.vector.tensor_copy(oev[0:DH, :, qs], o_ps[0:DH, :, qs])
        nc.vector.tensor_copy(drow[:, :, qs], o_ps[DH : DH + 1, :, qs])
        nc.vector.reciprocal_approx_fast(out=recip[:, :, qs], in_=drow[:, :, qs])
        for a in range(2):
            bc_ps = paQ.tile([P, 512], F32, tag="pa", name="bc_ps")
            nc.tensor.matmul(
                bc_ps[0:DH, :HB],
                lhsT=ones_f32[:],
                rhs=recip[:, a, qs],
                start=True,
                stop=True,
            )
            nc.vector.tensor_tensor(
                on[g, b][a * 64 : a * 64 + 64, qs],
                oev[0:DH, a, qs],
                bc_ps[0:DH, :HB],
                mybir.AluOpType.mult,
            )

    NH = cfg.dim // 512
    ot_half = {}

    def out_proj_chunk(bb, t, nh, scalar_evac=False, pool=None):
        nt = bb * (QB // 128) + t
        if pool is None:
            ps = paQ.tile([P, 512], F32, tag="pa", name="pc")
        else:
            # tail: borrow idle psS/psO banks for a deeper psum pipeline
            ps = pool.tile([P, 2, 512], F32, tag=("s" if pool is psS else "o"), name="pc")[:, 0, :]
        for kc in range(M_SLABS):
            nc.tensor.matmul(
                ps[:],
                lhsT=on[kc, bb][:, ts(t, 128)],
                rhs=wo_sb[:, kc, ts(nh, 512)],
                start=(kc == 0),
                stop=(kc == M_SLABS - 1),
            )
        # both nh halves share one [128, 1024] tile; a single row-block DMA
        # fires with the second half (fewer DMAs -> less sem/teardown cost)
        if nh == 0:
            ot = copool.tile([P, 1024], MD, tag="ot", name="ot")
            ot_half[nt] = ot
        else:
            ot = ot_half.pop(nt)
        if scalar_evac:
            nc.scalar.copy(ot[:, ts(nh, 512)], ps[:])
        else:
            nc.vector.tensor_copy(ot[:, ts(nh, 512)], ps[:])
        if nh == NH - 1:
            nc.sync.dma_start(out[ts(nt, 128), :], ot[:])

    # ---- emission schedule (b-major: both head-pairs of a block, then the
    # next block; out-proj chunks for block b become fillers two sweeps on) ----
    emit_qk(wk_sb, kt, 0, 0)
    emitted.add(("k", 0, 0))
    emit_qk(wq_sb, qt, 0, 0)
    emitted.add(("q", 0, 0))

    for b in range(1, NQB):
        pend.append((("k", 0, b), partial(emit_qk, wk_sb, kt, 0, b)))
    for b in range(1, NQB):
        pend.append((("q", 0, b), partial(emit_qk, wq_sb, qt, 0, b)))
    pend.append((("q", 1, 0), partial(emit_qk, wq_sb, qt, 1, 0)))
    for b in range(NQB):
        pend.append((("k", 1, b), partial(emit_qk, wk_sb, kt, 1, b)))
    for b in range(1, NQB):
        pend.append((("q", 1, b), partial(emit_qk, wq_sb, qt, 1, b)))

    for g in range(PAIRS):
        for b in range(NQB):
            first = b == 0 and g == 0
            last = b == NQB - 1 and g == PAIRS - 1
            o_ps_tail = attention(b, g, with_v=first, slots=not first, tail=last)
            if g == PAIRS - 1 and b < NQB - 1:
                for t in range(QB // 128):
                    for nh in range(NH):
                        pend_small.append(
                            (("o", b, t, nh), partial(out_proj_chunk, b, t, nh))
                        )
    while pend:
        fill_one()
    while pend_small:
        fill_one(pend_small)
    # tail block: normalize each query half, then immediately emit its two
    # row-block out-proj chunks so ready work is never queued behind the
    # other half's normalize chain; borrow idle psS banks (NOT psO — o_ps is
    # still live) for a deeper psum pipeline, and alternate evac engines
    # (ScalarE is idle after the last exp)
    i = 0
    for hx in range(2):
        tail_norm(o_ps_tail, PAIRS - 1, NQB - 1, hx)
        for t in range(2 * hx, 2 * hx + 2):
            for nh in range(NH):
                out_proj_chunk(
                    NQB - 1, t, nh, scalar_evac=(nh == 0),
                    pool=psS if i % 2 else None,
                )
                i += 1


def build_program(cfg, num_devices=N_CORES):
    nc = bacc.Bacc("TRN2", target_bir_lowering=False, debug=False, num_devices=num_devices)
    P = 128
    qkdt = F8 if cfg.qk_fp8 else cfg.mm_dt
    xT = nc.dram_tensor("xT", [P, cfg.kc, cfg.n], cfg.mm_dt, kind="ExternalInput").ap()
    xT8 = nc.dram_tensor("xT8", [P, cfg.kc, cfg.n], qkdt, kind="ExternalInput").ap()
    wq = nc.dram_tensor("wq", [P, cfg.kc, cfg.shard], qkdt, kind="ExternalInput").ap()
    wk = nc.dram_tensor("wk", [P, cfg.kc, cfg.shard], qkdt, kind="ExternalInput").ap()
    wv = nc.dram_tensor("wv", [P, cfg.kc, cfg.vw], cfg.mm_dt, kind="ExternalInput").ap()
    wo = nc.dram_tensor("wo", [P, cfg.shard // 128, cfg.dim], cfg.mm_dt, kind="ExternalInput").ap()
    out = nc.dram_tensor("out", [cfg.n, cfg.dim], cfg.mm_dt, kind="ExternalOutput").ap()
    with tile.TileContext(nc) as tc, ExitStack() as ctx:
        build_kernel(tc, ctx, cfg, xT, xT8, wq, wk, wv, wo, out)
    nc.compile()
    return nc


def shard_inputs(cfg, x, W_qkv, W_out, n_groups):
    """Build per-core input maps. Core c = (batch b, head-group g): c = b*n_groups + g."""
    import ml_dtypes

    f8 = ml_dtypes.float8_e4m3
    qk_dt = f8 if cfg.qk_fp8 else cfg.np_dt
    b_sz = x.shape[0]
    dim, hg, sh = cfg.dim, cfg.hg, cfg.shard
    xTs = []
    for b in range(b_sz):
        xt = np.ascontiguousarray(
            x[b].T.reshape(cfg.kc, 128, cfg.n).transpose(1, 0, 2)
        )
        xTs.append(xt)

    def wlayout(w):  # [dim, C] -> [128, kc, C]
        return np.ascontiguousarray(
            w.reshape(cfg.kc, 128, w.shape[1]).transpose(1, 0, 2)
        )

    in_maps = []
    for b in range(b_sz):
        for g in range(n_groups):
            wq = W_qkv[:, sh * g : sh * (g + 1)]
            wk = W_qkv[:, dim + sh * g : dim + sh * (g + 1)]
            wv_cols = W_qkv[:, 2 * dim + sh * g : 2 * dim + sh * (g + 1)]
            wv = np.zeros((dim, cfg.vw), np.float32)
            for h in range(hg):
                wv[:, h * (DH + 1) : h * (DH + 1) + DH] = wv_cols[:, h * DH : (h + 1) * DH]
            wo = W_out[sh * g : sh * (g + 1), :]
            wo_l = np.ascontiguousarray(
                wo.reshape(sh // 128, 128, dim).transpose(1, 0, 2)
            )
            in_maps.append(
                {
                    "xT": xTs[b].astype(cfg.np_dt),
                    "xT8": xTs[b].astype(qk_dt),
                    "wq": wlayout(wq).astype(qk_dt),
                    "wk": wlayout(wk).astype(qk_dt),
                    "wv": wlayout(wv).astype(cfg.np_dt),
                    "wo": wo_l.astype(cfg.np_dt),
                }
            )
    return in_maps


_NC_CACHE = {}


def kernel(x, W_qkv, W_out, b_out):
    x = np.asarray(x, np.float32)
    W_qkv = np.asarray(W_qkv, np.float32)
    W_out = np.asarray(W_out, np.float32)
    b_out = np.asarray(b_out, np.float32)
    cfg = FULL
    bsz = x.shape[0]
    n_groups = N_CORES // bsz

    if "nc" not in _NC_CACHE:
        _NC_CACHE["nc"] = build_program(cfg)
    nc = _NC_CACHE["nc"]

    in_maps = shard_inputs(cfg, x, W_qkv, W_out, n_groups)
    res = run_bass_kernel_spmd(nc, in_maps, list(range(N_CORES)))

    out = np.zeros((bsz, cfg.n, cfg.dim), np.float32)
    for b in range(bsz):
        for g in range(n_groups):
            out[b] += res.results[b * n_groups + g]["out"].astype(np.float32)
        out[b] += b_out
    return out
